# revision 1
# baseline (speedup 1.0000x reference)
import sys

sys.path.insert(0, "/opt/trn_rl_repo")
import numpy as np
import ml_dtypes
import concourse.bacc as bacc
import concourse.mybir as mybir
import concourse.tile as tile
from concourse.bass_utils import run_bass_kernel_spmd

F32R = mybir.dt.float32r
F32 = mybir.dt.float32
BF16 = mybir.dt.bfloat16
AF = mybir.ActivationFunctionType

B, S, D, H, DV = 2, 2048, 1024, 16, 64
NKT = 8     # k-tiles of 128 over D
NJ = 4      # query chunks of 512
NB = 16     # key blocks of 128
HPC = 4     # heads per core
DOFF = [0, 512, 1024, 1280]  # diag-pack column offsets (bank-aligned: dd2/dd3 share bank 2)
DW = [512, 384, 256, 128]    # diag-pack widths

_NC = None


def _build(debug=False):
    nc = bacc.Bacc(target_bir_lowering=False)
    xq = nc.dram_tensor("xq", [D, S], F32R, kind="ExternalInput")
    xk = nc.dram_tensor("xk", [D, S], F32R, kind="ExternalInput")
    xv = nc.dram_tensor("xv", [D, S], BF16, kind="ExternalInput")
    wq = nc.dram_tensor("wq", [D, 256], F32R, kind="ExternalInput")
    wk = nc.dram_tensor("wk", [D, 256], F32R, kind="ExternalInput")
    wv = nc.dram_tensor("wv", [D, 256], BF16, kind="ExternalInput")
    w0 = nc.dram_tensor("w0", [256, D], BF16, kind="ExternalInput")
    cm = nc.dram_tensor("cm", [4, 128, 512], F32R, kind="ExternalInput")
    yt = nc.dram_tensor("yt", [D, S], F32, kind="ExternalOutput")
    if debug:
        qt_d = nc.dram_tensor("qt_d", [2, 128, S], F32R, kind="ExternalOutput")
        kt_d = nc.dram_tensor("kt_d", [2, 128, S], F32R, kind="ExternalOutput")
        v_d = nc.dram_tensor("v_d", [128, NB, HPC, 65], F32R, kind="ExternalOutput")
        ot_d = nc.dram_tensor("ot_d", [HPC, 64, S], BF16, kind="ExternalOutput")

    with tile.TileContext(nc) as tc:
        with tc.tile_pool(name="pp", bufs=1) as pp:
            qt_sb = [pp.tile([128, S], F32R, name=f"qtsb{i}", tag=f"qtsb{i}") for i in range(2)]
            kt_sb = [pp.tile([128, S], F32R, name=f"ktsb{i}", tag=f"ktsb{i}") for i in range(2)]
            v_sb = pp.tile([128, NB, HPC, 65], F32R, name="vsb", tag="vsb")
            w0_sb = [pp.tile([64, D], BF16, name=f"w0sb{h}", tag=f"w0sb{h}") for h in range(HPC)]
            ot_sb = [pp.tile([64, S], BF16, name=f"otsb{h}", tag=f"otsb{h}") for h in range(HPC)]
            cm_sb = pp.tile([128, 4, 512], F32R, name="cmsb", tag="cmsb")
            ones65 = pp.tile([65, 64], F32R, name="ones65", tag="ones65")
            onestage = pp.tile([65, 64], F32, name="onestage", tag="onestage")
            vstage = pp.tile([128, NB, HPC], F32, name="vstage", tag="vstage")

            # constants + weights + cm on the ACT DMA queue
            for i in range(4):
                nc.scalar.dma_start(out=cm_sb[:, i, :], in_=cm[i, :, :])
            for h in range(HPC):
                nc.scalar.dma_start(out=w0_sb[h][:, :], in_=w0[64 * h:64 * h + 64, :])
            nc.vector.memset(onestage[64:65, :], 1.0)
            nc.vector.tensor_copy(ones65[64:65, :], onestage[64:65, :])
            nc.vector.memset(vstage[:, :, :], 1.0)
            nc.vector.tensor_copy(v_sb[:, :, :, 64], vstage[:, :, :])

            # ---- Phase A: projections (kt-outer, xv -> xq -> xk) ----
            with tc.tile_pool(name="wts", bufs=1) as wts, \
                 tc.tile_pool(name="xin", bufs=1) as xin, \
                 tc.tile_pool(name="psA", bufs=8, space="PSUM") as psA:
                wv_t, wq_t, wk_t = [], [], []
                for kt in range(NKT):
                    t = wts.tile([128, 256], BF16, name=f"wv{kt}", tag=f"wv{kt}")
                    nc.scalar.dma_start(out=t[:, :], in_=wv[128 * kt:128 * kt + 128, :])
                    wv_t.append(t)
                for kt in range(NKT):
                    t = wts.tile([128, 256], F32R, name=f"wq{kt}", tag=f"wq{kt}")
                    nc.scalar.dma_start(out=t[:, :], in_=wq[128 * kt:128 * kt + 128, :])
                    wq_t.append(t)
                for kt in range(NKT):
                    t = wts.tile([128, 256], F32R, name=f"wk{kt}", tag=f"wk{kt}")
                    nc.scalar.dma_start(out=t[:, :], in_=wk[128 * kt:128 * kt + 128, :])
                    wk_t.append(t)

                # xv (bf16) lands first so V blocks are ready when PV starts
                xv_t = []
                for kt in range(NKT):
                    t = xin.tile([128, S], BF16, name=f"xv{kt}", tag="xv", bufs=8)
                    nc.sync.dma_start(out=t[:, :], in_=xv[128 * kt:128 * kt + 128, :])
                    xv_t.append(t)
                xq_t = []
                for kt in range(NKT):
                    t = xin.tile([128, S], F32R, name=f"xq{kt}", tag="x", bufs=6)
                    nc.sync.dma_start(out=t[:, :], in_=xq[128 * kt:128 * kt + 128, :])
                    xq_t.append(t)
                xk_t = []
                for kt in range(NKT):
                    t = xin.tile([128, S], F32R, name=f"xk{kt}", tag="x", bufs=6)
                    nc.sync.dma_start(out=t[:, :], in_=xk[128 * kt:128 * kt + 128, :])
                    xk_t.append(t)

                # V projection: 2 waves x 8 st-groups, kt-outer within a wave
                for w in range(2):
                    vps = [psA.tile([128, HPC, 64], F32, name=f"vps{w}{g}", tag="pj")
                           for g in range(8)]
                    for kt in range(NKT):
                        for g in range(8):
                            st = 8 * w + g
                            nc.tensor.matmul(
                                vps[g][:, :, :],
                                xv_t[kt][:, 128 * st:128 * st + 128],
                                wv_t[kt][:, :],
                                start=(kt == 0), stop=(kt == NKT - 1))
                    for g in range(8):
                        nc.vector.tensor_copy(v_sb[:, 8 * w + g, :, 0:64], vps[g][:, :, :])

                # QT / KT: kt-outer, all 8 (p, jj) psum groups live
                for which, wt, xt, dst in (("q", wq_t, xq_t, qt_sb), ("k", wk_t, xk_t, kt_sb)):
                    qps = [psA.tile([128, 512], F32, name=f"{which}ps{i}", tag="pj")
                           for i in range(8)]
                    for kt in range(NKT):
                        for p in range(2):
                            for jj in range(4):
                                nc.tensor.matmul(
                                    qps[4 * p + jj][:, :],
                                    wt[kt][:, 128 * p:128 * p + 128],
                                    xt[kt][:, 512 * jj:512 * jj + 512],
                                    start=(kt == 0), stop=(kt == NKT - 1))
                    for p in range(2):
                        for jj in range(4):
                            nc.vector.tensor_copy(dst[p][:, 512 * jj:512 * jj + 512],
                                                  qps[4 * p + jj][:, :])

            # ---- Phase B/C interleaved: attention (j-outer) + out-proj ----
            with tc.tile_pool(name="pb", bufs=1) as pb, \
                 tc.tile_pool(name="psB", bufs=1, space="PSUM") as psB:

                pending = []

                def emit_norm(h, j, opsum):
                    # numerators rows 0:64, den row 64.  bcps is allocated
                    # while the rotation slot holds the already-normalized
                    # older opsum (opsum alloc comes after flush_norm), so the
                    # WAR is forward-only.
                    den = pb.tile([65, 512], F32R, name="den", tag="den", bufs=2)
                    nc.vector.tensor_copy(den[64:65, :], opsum[64:65, :])
                    bcps = psB.tile([64, 512], F32, name="bcps", tag="acc", bufs=2)
                    nc.tensor.matmul(bcps[:, :], ones65[64:65, :], den[64:65, :],
                                     start=True, stop=True)
                    rec = pb.tile([64, 512], F32, name="rec", tag="rec", bufs=2)
                    nc.vector.reciprocal_approx_fast(rec[:, :], bcps[:, :])
                    nc.vector.tensor_mul(ot_sb[h][:, 512 * j:512 * j + 512],
                                         opsum[0:64, :], rec[:, :])

                def flush_norm():
                    while pending:
                        emit_norm(*pending.pop(0))

                def emit_phase_c(j):
                    for e in range(8):
                        yps = psB.tile([128, 512], F32, name="yps", tag="acc", bufs=2)
                        for h in range(HPC):
                            nc.tensor.matmul(
                                yps[:, :],
                                w0_sb[h][:, 128 * e:128 * e + 128],
                                ot_sb[h][:, 512 * j:512 * j + 512],
                                start=(h == 0), stop=(h == HPC - 1))
                        ysb = pb.tile([128, 512], F32, name="ysb", tag="ysb", bufs=3)
                        nc.vector.tensor_copy(ysb[:, :], yps[:, :])
                        nc.sync.dma_start(out=yt[128 * e:128 * e + 128, 512 * j:512 * j + 512],
                                          in_=ysb[:, :])

                for j in range(NJ):
                    for h in range(HPC):
                        pair, pbase = h // 2, 64 * (h % 2)
                        offs = list(range(4 * j))
                        trips = [offs[t:t + 3] for t in range(0, len(offs), 3)] + ["diag"]
                        ntrip = len(trips)
                        st_tiles = {}

                        def emit_scores(t, trips=trips, st_tiles=st_tiles,
                                        pair=pair, pbase=pbase, j=j):
                            stile = psB.tile([128, 1536], F32, name="stile", tag="stile", bufs=2)
                            st_tiles[t] = stile
                            if trips[t] == "diag":
                                for dd in range(4):
                                    i = 4 * j + dd
                                    nc.tensor.matmul(
                                        stile[:, DOFF[dd]:DOFF[dd] + DW[dd]],
                                        kt_sb[pair][pbase:pbase + 64, 128 * i:128 * i + 128],
                                        qt_sb[pair][pbase:pbase + 64,
                                                    512 * j + 128 * dd:512 * j + 512],
                                        start=(dd != 3), stop=(dd != 2))
                            else:
                                for n, i in enumerate(trips[t]):
                                    nc.tensor.matmul(
                                        stile[:, 512 * n:512 * n + 512],
                                        kt_sb[pair][pbase:pbase + 64, 128 * i:128 * i + 128],
                                        qt_sb[pair][pbase:pbase + 64, 512 * j:512 * j + 512],
                                        start=True, stop=True)

                        emit_scores(0)
                        flush_norm()
                        if ntrip > 1:
                            emit_scores(1)
                        if h == 0 and j > 0:
                            emit_phase_c(j - 1)
                        opsum = psB.tile([128, 512], F32, name="opsum", tag="acc", bufs=2)
                        for t in range(ntrip):
                            ptt = pb.tile([128, 1536], F32R, name="ptt", tag="ptt", bufs=2)
                            if trips[t] == "diag":
                                nc.scalar.activation(ptt[:, 0:896], st_tiles[t][:, 0:896], AF.Exp)
                                nc.scalar.activation(ptt[:, 1024:1408],
                                                     st_tiles[t][:, 1024:1408], AF.Exp)
                            else:
                                width = 512 * len(trips[t])
                                nc.scalar.activation(ptt[:, 0:width], st_tiles[t][:, 0:width],
                                                     AF.Exp)
                            if trips[t] == "diag":
                                for dd in range(4):
                                    nc.vector.tensor_mul(
                                        ptt[:, DOFF[dd]:DOFF[dd] + 128],
                                        ptt[:, DOFF[dd]:DOFF[dd] + 128],
                                        cm_sb[:, dd, 128 * dd:128 * dd + 128])
                            if t + 2 < ntrip:
                                emit_scores(t + 2)
                            if trips[t] == "diag":
                                for dd in range(4):
                                    nc.tensor.matmul(
                                        opsum[0:65, 128 * dd:512],
                                        v_sb[:, 4 * j + dd, h, :],
                                        ptt[:, DOFF[dd]:DOFF[dd] + DW[dd]],
                                        start=(j == 0 and dd == 0), stop=(dd == 3))
                            else:
                                for n, i in enumerate(trips[t]):
                                    nc.tensor.matmul(
                                        opsum[0:65, :],
                                        v_sb[:, i, h, :],
                                        ptt[:, 512 * n:512 * n + 512],
                                        start=(t == 0 and n == 0), stop=False)
                        pending.append((h, j, opsum))
                flush_norm()
                emit_phase_c(NJ - 1)

                if debug:
                    for p in range(2):
                        nc.sync.dma_start(out=qt_d[p, :, :], in_=qt_sb[p][:, :])
                        nc.sync.dma_start(out=kt_d[p, :, :], in_=kt_sb[p][:, :])
                    nc.sync.dma_start(out=v_d[:, :, :, :], in_=v_sb[:, :, :, :])
                    for h in range(HPC):
                        nc.sync.dma_start(out=ot_d[h, :, :], in_=ot_sb[h][:, :])

    nc.compile()
    return nc


def _run(inputs, trace=False, debug=False):
    global _NC
    if _NC is None:
        _NC = _build(debug=debug)
    q = np.asarray(inputs["q"], dtype=np.float32)
    k = np.asarray(inputs["k"], dtype=np.float32)
    v = np.asarray(inputs["v"], dtype=np.float32)
    mask = np.asarray(inputs["mask"])
    w_query = np.asarray(inputs["w_query"], dtype=np.float32)
    w_key = np.asarray(inputs["w_key"], dtype=np.float32)
    w_value = np.asarray(inputs["w_value"], dtype=np.float32)
    w_0 = np.asarray(inputs["w_0"], dtype=np.float32)

    cmask = np.stack([
        np.ascontiguousarray(mask[0, 0, 0:512, 128 * i:128 * i + 128].T)
        for i in range(4)
    ]).astype(np.float32)
    xq_b = [np.ascontiguousarray(q[b].T) for b in range(B)]
    xk_b = [np.ascontiguousarray(k[b].T) for b in range(B)]
    xv_b = [np.ascontiguousarray(v[b].T).astype(ml_dtypes.bfloat16) for b in range(B)]

    in_maps = []
    for c in range(8):
        b, g = c // 4, c % 4
        sl = slice(256 * g, 256 * g + 256)
        in_maps.append({
            "xq": xq_b[b], "xk": xk_b[b], "xv": xv_b[b],
            "wq": np.ascontiguousarray(w_query[sl, :].T),
            "wk": np.ascontiguousarray(w_key[sl, :].T),
            "wv": np.ascontiguousarray(w_value[sl, :].T).astype(ml_dtypes.bfloat16),
            "w0": np.ascontiguousarray(w_0[:, sl].T).astype(ml_dtypes.bfloat16),
            "cm": cmask,
        })

    res = run_bass_kernel_spmd(_NC, in_maps, core_ids=list(range(8)), trace=trace)
    y = np.empty((B, S, D), dtype=np.float32)
    for b in range(B):
        acc = res.results[4 * b]["yt"].copy()
        for g in range(1, 4):
            acc += res.results[4 * b + g]["yt"]
        y[b] = acc.T
    if debug:
        return y, getattr(res, "exec_time_ns", None), res
    return y, getattr(res, "exec_time_ns", None)


def kernel(**inputs):
    return _run(inputs, trace=False)[0]



# revision 3
# speedup vs baseline: 1.0338x; 1.0338x over previous
import sys

sys.path.insert(0, "/opt/trn_rl_repo")
import numpy as np
import ml_dtypes
import concourse.bacc as bacc
import concourse.mybir as mybir
import concourse.tile as tile
from concourse.bass_utils import run_bass_kernel_spmd

F32R = mybir.dt.float32r
F32 = mybir.dt.float32
BF16 = mybir.dt.bfloat16
AF = mybir.ActivationFunctionType

B, S, D, H, DV = 2, 2048, 1024, 16, 64
NKT = 8     # k-tiles of 128 over D
NJ = 4      # query chunks of 512
NB = 16     # key blocks of 128
HPC = 4     # heads per core
DOFF = [0, 512, 1024, 1280]  # diag-pack column offsets (bank-aligned: dd2/dd3 share bank 2)
DW = [512, 384, 256, 128]    # diag-pack widths

_NC = None


def _build(debug=False):
    nc = bacc.Bacc(target_bir_lowering=False)
    xq = nc.dram_tensor("xq", [D, S], F32R, kind="ExternalInput")
    xk = nc.dram_tensor("xk", [D, S], F32R, kind="ExternalInput")
    xv = nc.dram_tensor("xv", [D, S], BF16, kind="ExternalInput")
    wq = nc.dram_tensor("wq", [D, 256], F32R, kind="ExternalInput")
    wk = nc.dram_tensor("wk", [D, 256], F32R, kind="ExternalInput")
    wv = nc.dram_tensor("wv", [D, 256], BF16, kind="ExternalInput")
    w0 = nc.dram_tensor("w0", [256, D], BF16, kind="ExternalInput")
    cm = nc.dram_tensor("cm", [4, 128, 512], F32R, kind="ExternalInput")
    yt = nc.dram_tensor("yt", [D, S], F32, kind="ExternalOutput")
    if debug:
        qt_d = nc.dram_tensor("qt_d", [2, 128, S], F32R, kind="ExternalOutput")
        kt_d = nc.dram_tensor("kt_d", [2, 128, S], F32R, kind="ExternalOutput")
        v_d = nc.dram_tensor("v_d", [128, NB, HPC, 65], F32R, kind="ExternalOutput")
        ot_d = nc.dram_tensor("ot_d", [HPC, 64, S], BF16, kind="ExternalOutput")

    with tile.TileContext(nc) as tc:
        with tc.tile_pool(name="pp", bufs=1) as pp:
            qt_sb = [pp.tile([128, S], F32R, name=f"qtsb{i}", tag=f"qtsb{i}") for i in range(2)]
            kt_sb = [pp.tile([128, S], F32R, name=f"ktsb{i}", tag=f"ktsb{i}") for i in range(2)]
            v_sb = pp.tile([128, NB, HPC, 65], F32R, name="vsb", tag="vsb")
            w0_sb = [pp.tile([64, D], BF16, name=f"w0sb{h}", tag=f"w0sb{h}") for h in range(HPC)]
            ot_sb = [pp.tile([64, S], BF16, name=f"otsb{h}", tag=f"otsb{h}") for h in range(HPC)]
            cm_sb = pp.tile([128, 4, 512], F32R, name="cmsb", tag="cmsb")
            ones65 = pp.tile([65, 64], F32R, name="ones65", tag="ones65")
            onestage = pp.tile([65, 64], F32, name="onestage", tag="onestage")
            vstage = pp.tile([128, NB, HPC], F32, name="vstage", tag="vstage")

            # constants + weights + cm on the ACT DMA queue
            for i in range(4):
                nc.scalar.dma_start(out=cm_sb[:, i, :], in_=cm[i, :, :])
            for h in range(HPC):
                nc.scalar.dma_start(out=w0_sb[h][:, :], in_=w0[64 * h:64 * h + 64, :])
            nc.vector.memset(onestage[64:65, :], 1.0)
            nc.vector.tensor_copy(ones65[64:65, :], onestage[64:65, :])
            nc.vector.memset(vstage[:, :, :], 1.0)
            nc.vector.tensor_copy(v_sb[:, :, :, 64], vstage[:, :, :])

            # ---- Phase A: projections (kt-outer, xv -> xq -> xk) ----
            with tc.tile_pool(name="wts", bufs=1) as wts, \
                 tc.tile_pool(name="xin", bufs=1) as xin, \
                 tc.tile_pool(name="psA", bufs=8, space="PSUM") as psA:
                wv_t, wq_t, wk_t = [], [], []
                for kt in range(NKT):
                    t = wts.tile([128, 256], BF16, name=f"wv{kt}", tag=f"wv{kt}")
                    nc.scalar.dma_start(out=t[:, :], in_=wv[128 * kt:128 * kt + 128, :])
                    wv_t.append(t)
                for kt in range(NKT):
                    t = wts.tile([128, 256], F32R, name=f"wq{kt}", tag=f"wq{kt}")
                    nc.scalar.dma_start(out=t[:, :], in_=wq[128 * kt:128 * kt + 128, :])
                    wq_t.append(t)
                for kt in range(NKT):
                    t = wts.tile([128, 256], F32R, name=f"wk{kt}", tag=f"wk{kt}")
                    nc.scalar.dma_start(out=t[:, :], in_=wk[128 * kt:128 * kt + 128, :])
                    wk_t.append(t)

                # xv (bf16) lands first so V blocks are ready when PV starts
                xv_t = []
                for kt in range(NKT):
                    t = xin.tile([128, S], BF16, name=f"xv{kt}", tag="xv", bufs=8)
                    nc.sync.dma_start(out=t[:, :], in_=xv[128 * kt:128 * kt + 128, :])
                    xv_t.append(t)
                xq_t = []
                for kt in range(NKT):
                    t = xin.tile([128, S], F32R, name=f"xq{kt}", tag="x", bufs=6)
                    nc.sync.dma_start(out=t[:, :], in_=xq[128 * kt:128 * kt + 128, :])
                    xq_t.append(t)
                xk_t = []
                for kt in range(NKT):
                    t = xin.tile([128, S], F32R, name=f"xk{kt}", tag="x", bufs=6)
                    nc.sync.dma_start(out=t[:, :], in_=xk[128 * kt:128 * kt + 128, :])
                    xk_t.append(t)

                # V projection: 2 waves x 8 st-groups, kt-outer within a wave
                for w in range(2):
                    vps = [psA.tile([128, HPC, 64], F32, name=f"vps{w}{g}", tag="pj")
                           for g in range(8)]
                    for kt in range(NKT):
                        for g in range(8):
                            st = 8 * w + g
                            nc.tensor.matmul(
                                vps[g][:, :, :],
                                xv_t[kt][:, 128 * st:128 * st + 128],
                                wv_t[kt][:, :],
                                start=(kt == 0), stop=(kt == NKT - 1))
                    for g in range(8):
                        nc.vector.tensor_copy(v_sb[:, 8 * w + g, :, 0:64], vps[g][:, :, :])

                # QT / KT: kt-outer, all 8 (p, jj) psum groups live
                for which, wt, xt, dst in (("q", wq_t, xq_t, qt_sb), ("k", wk_t, xk_t, kt_sb)):
                    qps = [psA.tile([128, 512], F32, name=f"{which}ps{i}", tag="pj")
                           for i in range(8)]
                    for kt in range(NKT):
                        for p in range(2):
                            for jj in range(4):
                                nc.tensor.matmul(
                                    qps[4 * p + jj][:, :],
                                    wt[kt][:, 128 * p:128 * p + 128],
                                    xt[kt][:, 512 * jj:512 * jj + 512],
                                    start=(kt == 0), stop=(kt == NKT - 1))
                    for p in range(2):
                        for jj in range(4):
                            nc.vector.tensor_copy(dst[p][:, 512 * jj:512 * jj + 512],
                                                  qps[4 * p + jj][:, :])

            # ---- Phase B/C interleaved: attention (j-outer) + out-proj ----
            with tc.tile_pool(name="pb", bufs=1) as pb, \
                 tc.tile_pool(name="psB", bufs=1, space="PSUM") as psB:

                pending = []

                def emit_norm(h, j, opsum):
                    # numerators rows 0:64, den row 64.  bcps is allocated
                    # while the rotation slot holds the already-normalized
                    # older opsum (opsum alloc comes after flush_norm), so the
                    # WAR is forward-only.
                    den = pb.tile([65, 512], F32R, name="den", tag="den", bufs=2)
                    nc.vector.tensor_copy(den[64:65, :], opsum[64:65, :])
                    bcps = psB.tile([64, 512], F32, name="bcps", tag="acc", bufs=2)
                    nc.tensor.matmul(bcps[:, :], ones65[64:65, :], den[64:65, :],
                                     start=True, stop=True)
                    rec = pb.tile([64, 512], F32, name="rec", tag="rec", bufs=2)
                    nc.vector.reciprocal_approx_fast(rec[:, :], bcps[:, :])
                    nc.vector.tensor_mul(ot_sb[h][:, 512 * j:512 * j + 512],
                                         opsum[0:64, :], rec[:, :])

                def flush_norm():
                    while pending:
                        emit_norm(*pending.pop(0))

                def emit_phase_c(j):
                    for e in range(8):
                        yps = psB.tile([128, 512], F32, name="yps", tag="acc", bufs=2)
                        for h in range(HPC):
                            nc.tensor.matmul(
                                yps[:, :],
                                w0_sb[h][:, 128 * e:128 * e + 128],
                                ot_sb[h][:, 512 * j:512 * j + 512],
                                start=(h == 0), stop=(h == HPC - 1))
                        ysb = pb.tile([128, 512], F32, name="ysb", tag="ysb", bufs=3)
                        nc.vector.tensor_copy(ysb[:, :], yps[:, :])
                        nc.sync.dma_start(out=yt[128 * e:128 * e + 128, 512 * j:512 * j + 512],
                                          in_=ysb[:, :])

                for j in range(NJ):
                    for h in range(HPC):
                        pair, pbase = h // 2, 64 * (h % 2)
                        offs = list(range(4 * j))
                        trips = [offs[t:t + 3] for t in range(0, len(offs), 3)] + ["diag"]
                        ntrip = len(trips)
                        st_tiles = {}

                        def emit_scores(t, trips=trips, st_tiles=st_tiles,
                                        pair=pair, pbase=pbase, j=j):
                            stile = psB.tile([128, 1536], F32, name="stile", tag="stile", bufs=2)
                            st_tiles[t] = stile
                            if trips[t] == "diag":
                                for dd in range(4):
                                    i = 4 * j + dd
                                    nc.tensor.matmul(
                                        stile[:, DOFF[dd]:DOFF[dd] + DW[dd]],
                                        kt_sb[pair][pbase:pbase + 64, 128 * i:128 * i + 128],
                                        qt_sb[pair][pbase:pbase + 64,
                                                    512 * j + 128 * dd:512 * j + 512],
                                        start=(dd != 3), stop=(dd != 2))
                            else:
                                for n, i in enumerate(trips[t]):
                                    nc.tensor.matmul(
                                        stile[:, 512 * n:512 * n + 512],
                                        kt_sb[pair][pbase:pbase + 64, 128 * i:128 * i + 128],
                                        qt_sb[pair][pbase:pbase + 64, 512 * j:512 * j + 512],
                                        start=True, stop=True)

                        emit_scores(0)
                        flush_norm()
                        if ntrip > 1:
                            emit_scores(1)
                        if h == 0 and j > 0:
                            emit_phase_c(j - 1)
                        opsum = psB.tile([128, 512], F32, name="opsum", tag="acc", bufs=2)
                        for t in range(ntrip):
                            ptt = pb.tile([128, 1536], F32R, name="ptt", tag="ptt", bufs=2)
                            if trips[t] == "diag":
                                nc.scalar.activation(ptt[:, 0:896], st_tiles[t][:, 0:896], AF.Exp)
                                nc.scalar.activation(ptt[:, 1024:1408],
                                                     st_tiles[t][:, 1024:1408], AF.Exp)
                            else:
                                width = 512 * len(trips[t])
                                nc.scalar.activation(ptt[:, 0:width], st_tiles[t][:, 0:width],
                                                     AF.Exp)
                            if trips[t] == "diag":
                                for dd in range(4):
                                    nc.vector.tensor_mul(
                                        ptt[:, DOFF[dd]:DOFF[dd] + 128],
                                        ptt[:, DOFF[dd]:DOFF[dd] + 128],
                                        cm_sb[:, dd, 128 * dd:128 * dd + 128])
                            if t + 2 < ntrip:
                                emit_scores(t + 2)
                            if trips[t] == "diag":
                                for dd in range(4):
                                    nc.tensor.matmul(
                                        opsum[0:65, 128 * dd:512],
                                        v_sb[:, 4 * j + dd, h, :],
                                        ptt[:, DOFF[dd]:DOFF[dd] + DW[dd]],
                                        start=(j == 0 and dd == 0), stop=(dd == 3))
                            else:
                                for n, i in enumerate(trips[t]):
                                    nc.tensor.matmul(
                                        opsum[0:65, :],
                                        v_sb[:, i, h, :],
                                        ptt[:, 512 * n:512 * n + 512],
                                        start=(t == 0 and n == 0), stop=False)
                        pending.append((h, j, opsum))
                flush_norm()
                emit_phase_c(NJ - 1)

                if debug:
                    for p in range(2):
                        nc.sync.dma_start(out=qt_d[p, :, :], in_=qt_sb[p][:, :])
                        nc.sync.dma_start(out=kt_d[p, :, :], in_=kt_sb[p][:, :])
                    nc.sync.dma_start(out=v_d[:, :, :, :], in_=v_sb[:, :, :, :])
                    for h in range(HPC):
                        nc.sync.dma_start(out=ot_d[h, :, :], in_=ot_sb[h][:, :])

    nc.compile()
    return nc


def _run(inputs, trace=False, debug=False, tmpdir=None):
    global _NC
    if _NC is None:
        _NC = _build(debug=debug)
    q = np.asarray(inputs["q"], dtype=np.float32)
    k = np.asarray(inputs["k"], dtype=np.float32)
    v = np.asarray(inputs["v"], dtype=np.float32)
    mask = np.asarray(inputs["mask"])
    w_query = np.asarray(inputs["w_query"], dtype=np.float32)
    w_key = np.asarray(inputs["w_key"], dtype=np.float32)
    w_value = np.asarray(inputs["w_value"], dtype=np.float32)
    w_0 = np.asarray(inputs["w_0"], dtype=np.float32)

    cmask = np.stack([
        np.ascontiguousarray(mask[0, 0, 0:512, 128 * i:128 * i + 128].T)
        for i in range(4)
    ]).astype(np.float32)
    xq_b = [np.ascontiguousarray(q[b].T) for b in range(B)]
    xk_b = [np.ascontiguousarray(k[b].T) for b in range(B)]
    xv_b = [np.ascontiguousarray(v[b].T).astype(ml_dtypes.bfloat16) for b in range(B)]

    in_maps = []
    for c in range(8):
        b, g = c // 4, c % 4
        sl = slice(256 * g, 256 * g + 256)
        in_maps.append({
            "xq": xq_b[b], "xk": xk_b[b], "xv": xv_b[b],
            "wq": np.ascontiguousarray(w_query[sl, :].T),
            "wk": np.ascontiguousarray(w_key[sl, :].T),
            "wv": np.ascontiguousarray(w_value[sl, :].T).astype(ml_dtypes.bfloat16),
            "w0": np.ascontiguousarray(w_0[:, sl].T).astype(ml_dtypes.bfloat16),
            "cm": cmask,
        })

    res = run_bass_kernel_spmd(_NC, in_maps, core_ids=list(range(8)), trace=trace,
                               tmpdir=tmpdir)
    y = np.empty((B, S, D), dtype=np.float32)
    for b in range(B):
        acc = res.results[4 * b]["yt"].copy()
        for g in range(1, 4):
            acc += res.results[4 * b + g]["yt"]
        y[b] = acc.T
    if debug:
        return y, getattr(res, "exec_time_ns", None), res
    return y, getattr(res, "exec_time_ns", None)


def kernel(**inputs):
    return _run(inputs, trace=False)[0]



# revision 4
# speedup vs baseline: 1.5137x; 1.4642x over previous
import sys

sys.path.insert(0, "/opt/trn_rl_repo")
import numpy as np
import ml_dtypes
import concourse.bacc as bacc
import concourse.mybir as mybir
import concourse.tile as tile
from concourse.bass_utils import run_bass_kernel_spmd

F32R = mybir.dt.float32r
F32 = mybir.dt.float32
BF16 = mybir.dt.bfloat16
AF = mybir.ActivationFunctionType

B, S, D, H, DV = 2, 2048, 1024, 16, 64
NKT = 8     # k-tiles of 128 over D
NJ = 4      # query chunks of 512
NB = 16     # key blocks of 128
HPC = 4     # heads per core
DOFF = [0, 512, 1024, 1280]  # diag-pack column offsets (bank-aligned: dd2/dd3 share bank 2)
DW = [512, 384, 256, 128]    # diag-pack widths

_NC = None


def _build(debug=False):
    nc = bacc.Bacc(target_bir_lowering=False)
    xq = nc.dram_tensor("xq", [D, S], BF16, kind="ExternalInput")
    xk = nc.dram_tensor("xk", [D, S], BF16, kind="ExternalInput")
    xv = nc.dram_tensor("xv", [D, S], BF16, kind="ExternalInput")
    wq = nc.dram_tensor("wq", [D, 256], BF16, kind="ExternalInput")
    wk = nc.dram_tensor("wk", [D, 256], BF16, kind="ExternalInput")
    wv = nc.dram_tensor("wv", [D, 256], BF16, kind="ExternalInput")
    w0 = nc.dram_tensor("w0", [256, D], BF16, kind="ExternalInput")
    cm = nc.dram_tensor("cm", [4, 128, 512], BF16, kind="ExternalInput")
    yt = nc.dram_tensor("yt", [D, S], BF16, kind="ExternalOutput")

    with tile.TileContext(nc) as tc:
        with tc.tile_pool(name="pp", bufs=1) as pp:
            qt_sb = [pp.tile([128, S], BF16, name=f"qtsb{i}", tag=f"qtsb{i}") for i in range(2)]
            kt_sb = [pp.tile([128, S], BF16, name=f"ktsb{i}", tag=f"ktsb{i}") for i in range(2)]
            v_sb = pp.tile([128, NB, HPC, 65], BF16, name="vsb", tag="vsb")
            w0_sb = [pp.tile([128, D], BF16, name=f"w0sb{p}", tag=f"w0sb{p}") for p in range(2)]
            ot_sb = [pp.tile([128, S], BF16, name=f"otsb{p}", tag=f"otsb{p}") for p in range(2)]
            cm_sb = pp.tile([128, 4, 512], BF16, name="cmsb", tag="cmsb")
            ones65 = pp.tile([65, 64], F32R, name="ones65", tag="ones65")
            onestage = pp.tile([65, 64], F32, name="onestage", tag="onestage")
            vstage = pp.tile([128, NB, HPC], BF16, name="vstage", tag="vstage")

            # constants + weights + cm on the GPSIMD DMA queue (keep ACT free)
            for h in range(2):
                nc.gpsimd.dma_start(out=w0_sb[h][:, :], in_=w0[128 * h:128 * h + 128, :])
            for i in range(4):
                nc.gpsimd.dma_start(out=cm_sb[:, i, :], in_=cm[i, :, :])
            nc.vector.memset(onestage[64:65, :], 1.0)
            nc.vector.tensor_copy(ones65[64:65, :], onestage[64:65, :])
            nc.vector.memset(vstage[:, :, :], 1.0)
            nc.vector.tensor_copy(v_sb[:, :, :, 64], vstage[:, :, :])

            # ---- Phase A: projections (kt-outer, xv -> xq -> xk) ----
            with tc.tile_pool(name="wts", bufs=1) as wts, \
                 tc.tile_pool(name="xin", bufs=1) as xin, \
                 tc.tile_pool(name="psA", bufs=8, space="PSUM") as psA:
                wv_t, wq_t, wk_t = [], [], []
                for kt in range(NKT):
                    t = wts.tile([128, 256], BF16, name=f"wv{kt}", tag=f"wv{kt}")
                    nc.gpsimd.dma_start(out=t[:, :], in_=wv[128 * kt:128 * kt + 128, :])
                    wv_t.append(t)
                for kt in range(NKT):
                    t = wts.tile([128, 256], BF16, name=f"wq{kt}", tag=f"wq{kt}")
                    nc.gpsimd.dma_start(out=t[:, :], in_=wq[128 * kt:128 * kt + 128, :])
                    wq_t.append(t)
                for kt in range(NKT):
                    t = wts.tile([128, 256], BF16, name=f"wk{kt}", tag=f"wk{kt}")
                    nc.gpsimd.dma_start(out=t[:, :], in_=wk[128 * kt:128 * kt + 128, :])
                    wk_t.append(t)

                # xv (bf16) lands first so V blocks are ready when PV starts
                xv_t = []
                for kt in range(NKT):
                    t = xin.tile([128, S], BF16, name=f"xv{kt}", tag="xv", bufs=8)
                    nc.sync.dma_start(out=t[:, :], in_=xv[128 * kt:128 * kt + 128, :])
                    xv_t.append(t)
                xq_t = []
                for kt in range(NKT):
                    t = xin.tile([128, S], BF16, name=f"xq{kt}", tag="x", bufs=16)
                    nc.sync.dma_start(out=t[:, :], in_=xq[128 * kt:128 * kt + 128, :])
                    xq_t.append(t)
                xk_t = []
                for kt in range(NKT):
                    t = xin.tile([128, S], BF16, name=f"xk{kt}", tag="x", bufs=16)
                    nc.sync.dma_start(out=t[:, :], in_=xk[128 * kt:128 * kt + 128, :])
                    xk_t.append(t)

                # V projection: 2 waves x 8 st-groups, kt-outer within a wave
                for w in range(2):
                    vps = [psA.tile([128, HPC, 64], F32, name=f"vps{w}{g}", tag="pj")
                           for g in range(8)]
                    for kt in range(NKT):
                        for g in range(8):
                            st = 8 * w + g
                            nc.tensor.matmul(
                                vps[g][:, :, :],
                                xv_t[kt][:, 128 * st:128 * st + 128],
                                wv_t[kt][:, :],
                                start=(kt == 0), stop=(kt == NKT - 1))
                    for g in range(8):
                        nc.vector.tensor_copy(v_sb[:, 8 * w + g, :, 0:64], vps[g][:, :, :])

                # QT / KT: kt-outer, all 8 (p, jj) psum groups live
                for which, wt, xt, dst in (("q", wq_t, xq_t, qt_sb), ("k", wk_t, xk_t, kt_sb)):
                    qps = [psA.tile([128, 512], F32, name=f"{which}ps{i}", tag="pj")
                           for i in range(8)]
                    for kt in range(NKT):
                        for p in range(2):
                            for jj in range(4):
                                nc.tensor.matmul(
                                    qps[4 * p + jj][:, :],
                                    wt[kt][:, 128 * p:128 * p + 128],
                                    xt[kt][:, 512 * jj:512 * jj + 512],
                                    start=(kt == 0), stop=(kt == NKT - 1))
                    for p in range(2):
                        for jj in range(4):
                            nc.vector.tensor_copy(dst[p][:, 512 * jj:512 * jj + 512],
                                                  qps[4 * p + jj][:, :])

            # ---- Phase B/C interleaved: attention (j-outer) + out-proj ----
            with tc.tile_pool(name="pb", bufs=1) as pb, \
                 tc.tile_pool(name="psB", bufs=1, space="PSUM") as psB:

                pending = []

                def emit_norm(h, j, opsum):
                    # numerators rows 0:64, den row 64.  bcps is allocated
                    # while the rotation slot holds the already-normalized
                    # older opsum (opsum alloc comes after flush_norm), so the
                    # WAR is forward-only.
                    den = pb.tile([65, 512], F32R, name="den", tag="den", bufs=2)
                    nc.vector.tensor_copy(den[64:65, :], opsum[64:65, :])
                    bcps = psB.tile([64, 512], F32, name="bcps", tag="acc", bufs=2)
                    nc.tensor.matmul(bcps[:, :], ones65[64:65, :], den[64:65, :],
                                     start=True, stop=True)
                    rec = pb.tile([64, 512], F32, name="rec", tag="rec", bufs=2)
                    nc.vector.reciprocal_approx_fast(rec[:, :], bcps[:, :])
                    nc.vector.tensor_mul(
                        ot_sb[h // 2][64 * (h % 2):64 * (h % 2) + 64,
                                      512 * j:512 * j + 512],
                        opsum[0:64, :], rec[:, :])

                def flush_norm():
                    while pending:
                        emit_norm(*pending.pop(0))

                def emit_phase_c(j):
                    for e in range(8):
                        yps = psB.tile([128, 512], F32, name="yps", tag="acc", bufs=2)
                        for p in range(2):
                            nc.tensor.matmul(
                                yps[:, :],
                                w0_sb[p][:, 128 * e:128 * e + 128],
                                ot_sb[p][:, 512 * j:512 * j + 512],
                                start=(p == 0), stop=(p == 1))
                        ysb = pb.tile([128, 512], BF16, name="ysb", tag="ysb", bufs=3)
                        nc.vector.tensor_copy(ysb[:, :], yps[:, :])
                        nc.gpsimd.dma_start(out=yt[128 * e:128 * e + 128, 512 * j:512 * j + 512],
                                            in_=ysb[:, :])

                for j in range(NJ):
                    for h in range(HPC):
                        pair, pbase = h // 2, 64 * (h % 2)
                        offs = list(range(4 * j))
                        trips = [offs[t:t + 3] for t in range(0, len(offs), 3)] + ["diag"]
                        ntrip = len(trips)
                        st_tiles = {}

                        def emit_scores(t, trips=trips, st_tiles=st_tiles,
                                        pair=pair, pbase=pbase, j=j):
                            stile = psB.tile([128, 1536], F32, name="stile", tag="stile", bufs=2)
                            st_tiles[t] = stile
                            if trips[t] == "diag":
                                for dd in range(4):
                                    i = 4 * j + dd
                                    nc.tensor.matmul(
                                        stile[:, DOFF[dd]:DOFF[dd] + DW[dd]],
                                        kt_sb[pair][pbase:pbase + 64, 128 * i:128 * i + 128],
                                        qt_sb[pair][pbase:pbase + 64,
                                                    512 * j + 128 * dd:512 * j + 512],
                                        start=(dd != 3), stop=(dd != 2))
                            else:
                                for n, i in enumerate(trips[t]):
                                    nc.tensor.matmul(
                                        stile[:, 512 * n:512 * n + 512],
                                        kt_sb[pair][pbase:pbase + 64, 128 * i:128 * i + 128],
                                        qt_sb[pair][pbase:pbase + 64, 512 * j:512 * j + 512],
                                        start=True, stop=True)

                        emit_scores(0)
                        flush_norm()
                        if ntrip > 1:
                            emit_scores(1)
                        if h == 0 and j > 0:
                            emit_phase_c(j - 1)
                        opsum = psB.tile([128, 512], F32, name="opsum", tag="acc", bufs=2)
                        for t in range(ntrip):
                            ptt = pb.tile([128, 1536], BF16, name="ptt", tag="ptt", bufs=3)
                            if trips[t] == "diag":
                                # one wide exp covering the 896:1024 gap (cols
                                # there are stale psum, never read by PV)
                                nc.scalar.activation(ptt[:, 0:1408], st_tiles[t][:, 0:1408],
                                                     AF.Exp)
                            else:
                                width = 512 * len(trips[t])
                                nc.scalar.activation(ptt[:, 0:width], st_tiles[t][:, 0:width],
                                                     AF.Exp)
                            if trips[t] == "diag":
                                for dd in range(4):
                                    nc.vector.tensor_mul(
                                        ptt[:, DOFF[dd]:DOFF[dd] + 128],
                                        ptt[:, DOFF[dd]:DOFF[dd] + 128],
                                        cm_sb[:, dd, 128 * dd:128 * dd + 128])
                            if t + 2 < ntrip:
                                emit_scores(t + 2)
                            if trips[t] == "diag":
                                for dd in range(4):
                                    nc.tensor.matmul(
                                        opsum[0:65, 128 * dd:512],
                                        v_sb[:, 4 * j + dd, h, :],
                                        ptt[:, DOFF[dd]:DOFF[dd] + DW[dd]],
                                        start=(j == 0 and dd == 0), stop=(dd == 3))
                            else:
                                for n, i in enumerate(trips[t]):
                                    nc.tensor.matmul(
                                        opsum[0:65, :],
                                        v_sb[:, i, h, :],
                                        ptt[:, 512 * n:512 * n + 512],
                                        start=(t == 0 and n == 0), stop=False)
                        pending.append((h, j, opsum))
                flush_norm()
                emit_phase_c(NJ - 1)

    nc.compile()
    return nc


def _run(inputs, trace=False, debug=False, tmpdir=None):
    global _NC
    if _NC is None:
        _NC = _build(debug=debug)
    q = np.asarray(inputs["q"], dtype=np.float32)
    k = np.asarray(inputs["k"], dtype=np.float32)
    v = np.asarray(inputs["v"], dtype=np.float32)
    mask = np.asarray(inputs["mask"])
    w_query = np.asarray(inputs["w_query"], dtype=np.float32)
    w_key = np.asarray(inputs["w_key"], dtype=np.float32)
    w_value = np.asarray(inputs["w_value"], dtype=np.float32)
    w_0 = np.asarray(inputs["w_0"], dtype=np.float32)

    cmask = np.stack([
        np.ascontiguousarray(mask[0, 0, 0:512, 128 * i:128 * i + 128].T)
        for i in range(4)
    ]).astype(ml_dtypes.bfloat16)
    xq_b = [np.ascontiguousarray(q[b].T).astype(ml_dtypes.bfloat16) for b in range(B)]
    xk_b = [np.ascontiguousarray(k[b].T).astype(ml_dtypes.bfloat16) for b in range(B)]
    xv_b = [np.ascontiguousarray(v[b].T).astype(ml_dtypes.bfloat16) for b in range(B)]

    in_maps = []
    for c in range(8):
        b, g = c // 4, c % 4
        sl = slice(256 * g, 256 * g + 256)
        in_maps.append({
            "xq": xq_b[b], "xk": xk_b[b], "xv": xv_b[b],
            "wq": np.ascontiguousarray(w_query[sl, :].T).astype(ml_dtypes.bfloat16),
            "wk": np.ascontiguousarray(w_key[sl, :].T).astype(ml_dtypes.bfloat16),
            "wv": np.ascontiguousarray(w_value[sl, :].T).astype(ml_dtypes.bfloat16),
            "w0": np.ascontiguousarray(w_0[:, sl].T).astype(ml_dtypes.bfloat16),
            "cm": cmask,
        })

    res = run_bass_kernel_spmd(_NC, in_maps, core_ids=list(range(8)), trace=trace,
                               tmpdir=tmpdir)
    y = np.empty((B, S, D), dtype=np.float32)
    for b in range(B):
        acc = res.results[4 * b]["yt"].astype(np.float32)
        for g in range(1, 4):
            acc += res.results[4 * b + g]["yt"].astype(np.float32)
        y[b] = acc.T
    if debug:
        return y, getattr(res, "exec_time_ns", None), res
    return y, getattr(res, "exec_time_ns", None)


def kernel(**inputs):
    return _run(inputs, trace=False)[0]


# revision 10
# speedup vs baseline: 1.5460x; 1.0213x over previous
import sys

sys.path.insert(0, "/opt/trn_rl_repo")
import numpy as np
import ml_dtypes
import concourse.bass as bass
import concourse.bacc as bacc
import concourse.mybir as mybir
import concourse.tile as tile
from concourse.bass_utils import run_bass_kernel_spmd

F32R = mybir.dt.float32r
F32 = mybir.dt.float32
BF16 = mybir.dt.bfloat16
F16 = mybir.dt.float16
AF = mybir.ActivationFunctionType

B, S, D, H, DV = 2, 2048, 1024, 16, 64
NKT = 8     # k-tiles of 128 over D
NJ = 4      # query chunks of 512
NB = 16     # key blocks of 128
HPC = 4     # heads per core
DOFF = [0, 512, 1024, 1280]  # diag-pack column offsets (bank-aligned: dd2/dd3 share bank 2)
DW = [512, 384, 256, 128]    # diag-pack widths

_NC = None


def _build(debug=False):
    nc = bacc.Bacc(target_bir_lowering=False)
    xq = nc.dram_tensor("xq", [D, S], F16, kind="ExternalInput")
    xk = nc.dram_tensor("xk", [D, S], F16, kind="ExternalInput")
    xv = nc.dram_tensor("xv", [D, S], BF16, kind="ExternalInput")
    wq = nc.dram_tensor("wq", [D, 256], F16, kind="ExternalInput")
    wk = nc.dram_tensor("wk", [D, 256], F16, kind="ExternalInput")
    wv = nc.dram_tensor("wv", [D, 256], BF16, kind="ExternalInput")
    w0 = nc.dram_tensor("w0", [256, D], F16, kind="ExternalInput")
    cm = nc.dram_tensor("cm", [4, 128, 512], BF16, kind="ExternalInput")
    yt = nc.dram_tensor("yt", [D, S], F16, kind="ExternalOutput")

    with tile.TileContext(nc) as tc:
        with tc.tile_pool(name="pp", bufs=1) as pp:
            qt_sb = [pp.tile([128, S], F16, name=f"qtsb{i}", tag=f"qtsb{i}") for i in range(2)]
            kt_sb = [pp.tile([128, S], F16, name=f"ktsb{i}", tag=f"ktsb{i}") for i in range(2)]
            v_sb = pp.tile([128, NB, HPC, 65], BF16, name="vsb", tag="vsb")
            w0_sb = [pp.tile([128, D], F16, name=f"w0sb{p}", tag=f"w0sb{p}") for p in range(2)]
            ot_sb = [pp.tile([128, S], F16, name=f"otsb{p}", tag=f"otsb{p}") for p in range(2)]
            cm_sb = pp.tile([128, 4, 512], BF16, name="cmsb", tag="cmsb")
            ones65 = pp.tile([65, 64], F32R, name="ones65", tag="ones65")
            onestage = pp.tile([65, 64], F32, name="onestage", tag="onestage")
            vstage = pp.tile([128, NB, HPC], BF16, name="vstage", tag="vstage")

            # weights first (V proj starts earliest), then cm/w0 (phase B/C)
            wv_t, wq_t, wk_t = [], [], []
            for kt in range(NKT):
                t = pp.tile([128, 256], BF16, name=f"wv{kt}", tag=f"wv{kt}")
                nc.gpsimd.dma_start(out=t[:, :], in_=wv[128 * kt:128 * kt + 128, :])
                wv_t.append(t)
            for kt in range(NKT):
                t = pp.tile([128, 256], F16, name=f"wq{kt}", tag=f"wq{kt}")
                nc.gpsimd.dma_start(out=t[:, :], in_=wq[128 * kt:128 * kt + 128, :])
                wq_t.append(t)
            for kt in range(NKT):
                t = pp.tile([128, 256], F16, name=f"wk{kt}", tag=f"wk{kt}")
                nc.gpsimd.dma_start(out=t[:, :], in_=wk[128 * kt:128 * kt + 128, :])
                wk_t.append(t)
            for i in range(4):
                nc.gpsimd.dma_start(out=cm_sb[:, i, :], in_=cm[i, :, :])
            for p in range(2):
                nc.gpsimd.dma_start(out=w0_sb[p][:, :], in_=w0[128 * p:128 * p + 128, :])
            nc.vector.memset(onestage[64:65, :], 1.0)
            nc.vector.tensor_copy(ones65[64:65, :], onestage[64:65, :])
            nc.vector.memset(vstage[:, :, :], 1.0)
            nc.vector.tensor_copy(v_sb[:, :, :, 64], vstage[:, :, :])

            # ---- Phase A: projections (kt-outer, xv -> xq -> xk) ----
            with tc.tile_pool(name="xin", bufs=1) as xin, \
                 tc.tile_pool(name="psA", bufs=8, space="PSUM") as psA:
                # xv + xq on the sync queue, xk on the scalar queue (idle in
                # phase A) so input bandwidth is not queue-limited
                xv_t = []
                for kt in range(NKT):
                    t = xin.tile([128, S], BF16, name=f"xv{kt}", tag="xv", bufs=8)
                    nc.sync.dma_start(out=t[:, :], in_=xv[128 * kt:128 * kt + 128, :])
                    xv_t.append(t)
                xq_t = []
                for kt in range(NKT):
                    t = xin.tile([128, S], F16, name=f"xq{kt}", tag="x", bufs=16)
                    nc.sync.dma_start(out=t[:, :], in_=xq[128 * kt:128 * kt + 128, :])
                    xq_t.append(t)
                xk_t = []
                for kt in range(NKT):
                    t = xin.tile([128, S], F16, name=f"xk{kt}", tag="x", bufs=16)
                    nc.scalar.dma_start(out=t[:, :], in_=xk[128 * kt:128 * kt + 128, :])
                    xk_t.append(t)

                # V projection: 2 waves x 8 st-groups, kt-outer within a wave
                for w in range(2):
                    vps = [psA.tile([128, HPC, 64], F32, name=f"vps{w}{g}", tag="pj")
                           for g in range(8)]
                    for kt in range(NKT):
                        for g in range(8):
                            st = 8 * w + g
                            nc.tensor.matmul(
                                vps[g][:, :, :],
                                xv_t[kt][:, 128 * st:128 * st + 128],
                                wv_t[kt][:, :],
                                start=(kt == 0), stop=(kt == NKT - 1))
                    for g in range(8):
                        nc.vector.tensor_copy(v_sb[:, 8 * w + g, :, 0:64], vps[g][:, :, :])

                # QT / KT: kt-outer, all 8 (p, jj) psum groups live
                for which, wt, xt, dst in (("q", wq_t, xq_t, qt_sb), ("k", wk_t, xk_t, kt_sb)):
                    qps = [psA.tile([128, 512], F32, name=f"{which}ps{i}", tag="pj")
                           for i in range(8)]
                    for kt in range(NKT):
                        for p in range(2):
                            for jj in range(4):
                                nc.tensor.matmul(
                                    qps[4 * p + jj][:, :],
                                    wt[kt][:, 128 * p:128 * p + 128],
                                    xt[kt][:, 512 * jj:512 * jj + 512],
                                    start=(kt == 0), stop=(kt == NKT - 1))
                    for p in range(2):
                        for jj in range(4):
                            nc.vector.tensor_copy(dst[p][:, 512 * jj:512 * jj + 512],
                                                  qps[4 * p + jj][:, :])

            # ---- Phase B/C interleaved: attention (j-outer) + out-proj ----
            with tc.tile_pool(name="pb", bufs=1) as pb, \
                 tc.tile_pool(name="psB", bufs=1, space="PSUM") as psB:

                pending = []

                def emit_norm(h, j, opsum):
                    # numerators rows 0:64, den row 64; broadcast den via the
                    # ones65 matmul, reciprocal, then normalize into ot.
                    den = pb.tile([65, 512], F32R, name="den", tag="den", bufs=2)
                    nc.vector.tensor_copy(den[64:65, :], opsum[64:65, :])
                    bcps = psB.tile([64, 512], F32, name="bcps", tag="acc", bufs=2)
                    nc.tensor.matmul(bcps[:, :], ones65[64:65, :], den[64:65, :],
                                     start=True, stop=True)
                    rec = pb.tile([64, 512], F32, name="rec", tag="rec", bufs=2)
                    nc.vector.reciprocal_approx_fast(rec[:, :], bcps[:, :])
                    nc.vector.tensor_mul(
                        ot_sb[h // 2][64 * (h % 2):64 * (h % 2) + 64,
                                      512 * j:512 * j + 512],
                        opsum[0:64, :], rec[:, :])

                def flush_norm():
                    while pending:
                        emit_norm(*pending.pop(0))

                def emit_phase_c(j, es):
                    for e in es:
                        yps = psB.tile([128, 512], F32, name="yps", tag="acc", bufs=2)
                        for p in range(2):
                            nc.tensor.matmul(
                                yps[:, :],
                                w0_sb[p][:, 128 * e:128 * e + 128],
                                ot_sb[p][:, 512 * j:512 * j + 512],
                                start=(p == 0), stop=(p == 1))
                        ysb = pb.tile([128, 512], F16, name="ysb", tag="ysb", bufs=3)
                        nc.vector.tensor_copy(ysb[:, :], yps[:, :])
                        nc.gpsimd.dma_start(out=yt[128 * e:128 * e + 128, 512 * j:512 * j + 512],
                                            in_=ysb[:, :])

                for j in range(NJ):
                    for h in range(HPC):
                        pair, pbase = h // 2, 64 * (h % 2)
                        offs = list(range(4 * j))
                        trips = [offs[t:t + 3] for t in range(0, len(offs), 3)] + ["diag"]
                        ntrip = len(trips)
                        st_tiles = {}

                        def emit_scores(t, trips=trips, st_tiles=st_tiles,
                                        pair=pair, pbase=pbase, j=j):
                            stile = psB.tile([128, 1536], F32, name="stile", tag="stile", bufs=2)
                            st_tiles[t] = stile
                            if trips[t] == "diag":
                                for dd in range(4):
                                    i = 4 * j + dd
                                    nc.tensor.matmul(
                                        stile[:, DOFF[dd]:DOFF[dd] + DW[dd]],
                                        kt_sb[pair][pbase:pbase + 64, 128 * i:128 * i + 128],
                                        qt_sb[pair][pbase:pbase + 64,
                                                    512 * j + 128 * dd:512 * j + 512],
                                        start=(dd != 3), stop=(dd != 2))
                            else:
                                for n, i in enumerate(trips[t]):
                                    nc.tensor.matmul(
                                        stile[:, 512 * n:512 * n + 512],
                                        kt_sb[pair][pbase:pbase + 64, 128 * i:128 * i + 128],
                                        qt_sb[pair][pbase:pbase + 64, 512 * j:512 * j + 512],
                                        start=True, stop=True)

                        emit_scores(0)
                        flush_norm()
                        if ntrip > 1:
                            emit_scores(1)
                        if j > 0:
                            emit_phase_c(j - 1, [2 * h, 2 * h + 1])
                        opsum = psB.tile([128, 512], F32, name="opsum", tag="acc", bufs=2)
                        for t in range(ntrip):
                            ptt = pb.tile([128, 1536], BF16, name="ptt", tag="ptt", bufs=3)
                            if trips[t] == "diag":
                                # one wide exp covering the 896:1024 gap (cols
                                # there are stale psum, never read by PV)
                                nc.scalar.activation(ptt[:, 0:1408], st_tiles[t][:, 0:1408],
                                                     AF.Exp)
                            else:
                                width = 512 * len(trips[t])
                                nc.scalar.activation(ptt[:, 0:width], st_tiles[t][:, 0:width],
                                                     AF.Exp)
                            if trips[t] == "diag":
                                for dd in range(4):
                                    nc.vector.tensor_mul(
                                        ptt[:, DOFF[dd]:DOFF[dd] + 128],
                                        ptt[:, DOFF[dd]:DOFF[dd] + 128],
                                        cm_sb[:, dd, 128 * dd:128 * dd + 128])
                            if t + 2 < ntrip:
                                emit_scores(t + 2)
                            if trips[t] == "diag":
                                for dd in range(4):
                                    nc.tensor.matmul(
                                        opsum[0:65, 128 * dd:512],
                                        v_sb[:, 4 * j + dd, h, :],
                                        ptt[:, DOFF[dd]:DOFF[dd] + DW[dd]],
                                        start=(j == 0 and dd == 0), stop=(dd == 3))
                            else:
                                for n, i in enumerate(trips[t]):
                                    nc.tensor.matmul(
                                        opsum[0:65, :],
                                        v_sb[:, i, h, :],
                                        ptt[:, 512 * n:512 * n + 512],
                                        start=(t == 0 and n == 0), stop=False)
                        pending.append((h, j, opsum))
                flush_norm()
                emit_phase_c(NJ - 1, list(range(8)))

    nc.compile()
    return nc


def _run(inputs, trace=False, debug=False, tmpdir=None):
    global _NC
    if _NC is None:
        _NC = _build(debug=debug)
    q = np.asarray(inputs["q"], dtype=np.float32)
    k = np.asarray(inputs["k"], dtype=np.float32)
    v = np.asarray(inputs["v"], dtype=np.float32)
    mask = np.asarray(inputs["mask"])
    w_query = np.asarray(inputs["w_query"], dtype=np.float32)
    w_key = np.asarray(inputs["w_key"], dtype=np.float32)
    w_value = np.asarray(inputs["w_value"], dtype=np.float32)
    w_0 = np.asarray(inputs["w_0"], dtype=np.float32)

    cmask = np.stack([
        np.ascontiguousarray(mask[0, 0, 0:512, 128 * i:128 * i + 128].T)
        for i in range(4)
    ]).astype(ml_dtypes.bfloat16)
    xq_b = [np.ascontiguousarray(q[b].T).astype(np.float16) for b in range(B)]
    xk_b = [np.ascontiguousarray(k[b].T).astype(np.float16) for b in range(B)]
    xv_b = [np.ascontiguousarray(v[b].T).astype(ml_dtypes.bfloat16) for b in range(B)]

    in_maps = []
    for c in range(8):
        b, g = c // 4, c % 4
        sl = slice(256 * g, 256 * g + 256)
        in_maps.append({
            "xq": xq_b[b], "xk": xk_b[b], "xv": xv_b[b],
            "wq": np.ascontiguousarray(w_query[sl, :].T).astype(np.float16),
            "wk": np.ascontiguousarray(w_key[sl, :].T).astype(np.float16),
            "wv": np.ascontiguousarray(w_value[sl, :].T).astype(ml_dtypes.bfloat16),
            "w0": np.ascontiguousarray(w_0[:, sl].T).astype(np.float16),
            "cm": cmask,
        })

    res = run_bass_kernel_spmd(_NC, in_maps, core_ids=list(range(8)), trace=trace,
                               tmpdir=tmpdir)
    y = np.empty((B, S, D), dtype=np.float32)
    for b in range(B):
        acc = res.results[4 * b]["yt"].astype(np.float32)
        for g in range(1, 4):
            acc += res.results[4 * b + g]["yt"].astype(np.float32)
        y[b] = acc.T
    if debug:
        return y, getattr(res, "exec_time_ns", None), res
    return y, getattr(res, "exec_time_ns", None)


def kernel(**inputs):
    return _run(inputs, trace=False)[0]


# revision 18
# speedup vs baseline: 1.5885x; 1.0275x over previous
import sys

sys.path.insert(0, "/opt/trn_rl_repo")
import numpy as np
import ml_dtypes
import concourse.bass as bass
import concourse.bacc as bacc
import concourse.mybir as mybir
import concourse.tile as tile
from concourse.bass_utils import run_bass_kernel_spmd

F32R = mybir.dt.float32r
F32 = mybir.dt.float32
BF16 = mybir.dt.bfloat16
F16 = mybir.dt.float16
AF = mybir.ActivationFunctionType

B, S, D, H, DV = 2, 2048, 1024, 16, 64
NKT = 8     # k-tiles of 128 over D
NJ = 4      # query chunks of 512
NB = 16     # key blocks of 128
HPC = 4     # heads per core
DOFF = [0, 512, 1024, 1280]  # diag-pack column offsets (bank-aligned: dd2/dd3 share bank 2)
DW = [512, 384, 256, 128]    # diag-pack widths

_NC = None


def _build(debug=False):
    nc = bacc.Bacc(target_bir_lowering=False)
    xq = nc.dram_tensor("xq", [D, S], F16, kind="ExternalInput")
    xk = nc.dram_tensor("xk", [D, S], F16, kind="ExternalInput")
    xv = nc.dram_tensor("xv", [D, S], BF16, kind="ExternalInput")
    wq = nc.dram_tensor("wq", [D, 256], F16, kind="ExternalInput")
    wk = nc.dram_tensor("wk", [D, 256], F16, kind="ExternalInput")
    wv = nc.dram_tensor("wv", [D, 256], BF16, kind="ExternalInput")
    w0 = nc.dram_tensor("w0", [256, D], F16, kind="ExternalInput")
    cm = nc.dram_tensor("cm", [4, 128, 512], BF16, kind="ExternalInput")
    yt = nc.dram_tensor("yt", [D, S], F16, kind="ExternalOutput")

    with tile.TileContext(nc) as tc:
        with tc.tile_pool(name="pp", bufs=1) as pp:
            qt_sb = [pp.tile([128, S], F16, name=f"qtsb{i}", tag=f"qtsb{i}") for i in range(2)]
            kt_sb = [pp.tile([128, S], F16, name=f"ktsb{i}", tag=f"ktsb{i}") for i in range(2)]
            v_sb = pp.tile([128, NB, HPC, 65], BF16, name="vsb", tag="vsb")
            w0_sb = [pp.tile([128, D], F16, name=f"w0sb{p}", tag=f"w0sb{p}") for p in range(2)]
            ot_sb = [pp.tile([128, S], F16, name=f"otsb{p}", tag=f"otsb{p}") for p in range(2)]
            cm_sb = pp.tile([128, 4, 512], BF16, name="cmsb", tag="cmsb")
            ones65 = pp.tile([65, 64], F32R, name="ones65", tag="ones65")
            onestage = pp.tile([65, 64], F32, name="onestage", tag="onestage")
            vstage = pp.tile([128, NB, HPC], BF16, name="vstage", tag="vstage")

            # weights first (V proj starts earliest), then cm/w0 (phase B/C)
            wv_t, wq_t, wk_t = [], [], []
            for kt in range(NKT):
                t = pp.tile([128, 256], BF16, name=f"wv{kt}", tag=f"wv{kt}")
                nc.gpsimd.dma_start(out=t[:, :], in_=wv[128 * kt:128 * kt + 128, :])
                wv_t.append(t)
            for kt in range(NKT):
                t = pp.tile([128, 256], F16, name=f"wq{kt}", tag=f"wq{kt}")
                nc.gpsimd.dma_start(out=t[:, :], in_=wq[128 * kt:128 * kt + 128, :])
                wq_t.append(t)
            for kt in range(NKT):
                t = pp.tile([128, 256], F16, name=f"wk{kt}", tag=f"wk{kt}")
                nc.gpsimd.dma_start(out=t[:, :], in_=wk[128 * kt:128 * kt + 128, :])
                wk_t.append(t)
            for i in range(4):
                nc.gpsimd.dma_start(out=cm_sb[:, i, :], in_=cm[i, :, :])
            for p in range(2):
                nc.gpsimd.dma_start(out=w0_sb[p][:, :], in_=w0[128 * p:128 * p + 128, :])
            nc.vector.memset(onestage[64:65, :], 1.0)
            nc.vector.tensor_copy(ones65[64:65, :], onestage[64:65, :])
            nc.vector.memset(vstage[:, :, :], 1.0)
            nc.vector.tensor_copy(v_sb[:, :, :, 64], vstage[:, :, :])
            # preload the exp table set during phase A (ACT is idle there)
            expwarm = pp.tile([1, 64], F32, name="expwarm", tag="expwarm")
            nc.scalar.activation(expwarm[0:1, :], onestage[64:65, :], AF.Exp)

            # ---- Phase A: projections (kt-outer, xv -> xq -> xk) ----
            with tc.tile_pool(name="xin", bufs=1) as xin, \
                 tc.tile_pool(name="psA", bufs=8, space="PSUM") as psA:
                # xv + xq on the sync queue, xk on the scalar queue (idle in
                # phase A) so input bandwidth is not queue-limited
                xv_t = []
                for kt in range(NKT):
                    t = xin.tile([128, S], BF16, name=f"xv{kt}", tag="xv", bufs=8)
                    nc.sync.dma_start(out=t[:, :], in_=xv[128 * kt:128 * kt + 128, :])
                    xv_t.append(t)
                xq_t = []
                for kt in range(NKT):
                    t = xin.tile([128, S], F16, name=f"xq{kt}", tag="x", bufs=16)
                    nc.sync.dma_start(out=t[:, :], in_=xq[128 * kt:128 * kt + 128, :])
                    xq_t.append(t)
                xk_t = []
                for kt in range(NKT):
                    t = xin.tile([128, S], F16, name=f"xk{kt}", tag="x", bufs=16)
                    nc.sync.dma_start(out=t[:, :], in_=xk[128 * kt:128 * kt + 128, :])
                    xk_t.append(t)

                # V projection: 2 waves x 8 st-groups, kt-outer within a wave
                for w in range(2):
                    vps = [psA.tile([128, HPC, 64], F32, name=f"vps{w}{g}", tag="pj")
                           for g in range(8)]
                    for kt in range(NKT):
                        for g in range(8):
                            st = 8 * w + g
                            nc.tensor.matmul(
                                vps[g][:, :, :],
                                xv_t[kt][:, 128 * st:128 * st + 128],
                                wv_t[kt][:, :],
                                start=(kt == 0), stop=(kt == NKT - 1))
                    for g in range(8):
                        nc.vector.tensor_copy(v_sb[:, 8 * w + g, :, 0:64], vps[g][:, :, :])

                # QT / KT: kt-outer, all 8 (p, jj) psum groups live
                for which, wt, xt, dst in (("q", wq_t, xq_t, qt_sb), ("k", wk_t, xk_t, kt_sb)):
                    qps = [psA.tile([128, 512], F32, name=f"{which}ps{i}", tag="pj")
                           for i in range(8)]
                    for kt in range(NKT):
                        for p in range(2):
                            for jj in range(4):
                                nc.tensor.matmul(
                                    qps[4 * p + jj][:, :],
                                    wt[kt][:, 128 * p:128 * p + 128],
                                    xt[kt][:, 512 * jj:512 * jj + 512],
                                    start=(kt == 0), stop=(kt == NKT - 1))
                    for p in range(2):
                        for jj in range(4):
                            nc.vector.tensor_copy(dst[p][:, 512 * jj:512 * jj + 512],
                                                  qps[4 * p + jj][:, :])

            # ---- Phase B/C interleaved: attention (j-outer) + out-proj ----
            with tc.tile_pool(name="pb", bufs=1) as pb, \
                 tc.tile_pool(name="psB", bufs=1, space="PSUM") as psB:

                def emit_norm(h, j, opsum):
                    # numerators rows 0:64, den row 64; broadcast den via the
                    # ones65 matmul, reciprocal, then normalize into ot.
                    den = pb.tile([65, 512], F32R, name="den", tag="den", bufs=2)
                    nc.vector.tensor_copy(den[64:65, :], opsum[64:65, :])
                    bcps = psB.tile([64, 512], F32, name="bcps", tag="acc", bufs=2)
                    nc.tensor.matmul(bcps[:, :], ones65[64:65, :], den[64:65, :],
                                     start=True, stop=True)
                    rec = pb.tile([64, 512], F32, name="rec", tag="rec", bufs=2)
                    nc.vector.reciprocal_approx_fast(rec[:, :], bcps[:, :])
                    nc.vector.tensor_mul(
                        ot_sb[h // 2][64 * (h % 2):64 * (h % 2) + 64,
                                      512 * j:512 * j + 512],
                        opsum[0:64, :], rec[:, :])

                def emit_phase_c(j, es):
                    for e in es:
                        yps = psB.tile([128, 512], F32, name="yps", tag="acc", bufs=2)
                        for p in range(2):
                            nc.tensor.matmul(
                                yps[:, :],
                                w0_sb[p][:, 128 * e:128 * e + 128],
                                ot_sb[p][:, 512 * j:512 * j + 512],
                                start=(p == 0), stop=(p == 1))
                        ysb = pb.tile([128, 512], F16, name="ysb", tag="ysb", bufs=3)
                        nc.vector.tensor_copy(ysb[:, :], yps[:, :])
                        nc.gpsimd.dma_start(out=yt[128 * e:128 * e + 128, 512 * j:512 * j + 512],
                                            in_=ysb[:, :])

                for j in range(NJ):
                    for h in range(HPC):
                        pair, pbase = h // 2, 64 * (h % 2)
                        offs = list(range(4 * j))
                        trips = [offs[t:t + 3] for t in range(0, len(offs), 3)] + ["diag"]
                        ntrip = len(trips)
                        st_tiles = {}

                        def emit_scores(t, trips=trips, st_tiles=st_tiles,
                                        pair=pair, pbase=pbase, j=j):
                            stile = psB.tile([128, 1536], F32, name="stile", tag="stile", bufs=2)
                            st_tiles[t] = stile
                            if trips[t] == "diag":
                                for dd in range(4):
                                    i = 4 * j + dd
                                    nc.tensor.matmul(
                                        stile[:, DOFF[dd]:DOFF[dd] + DW[dd]],
                                        kt_sb[pair][pbase:pbase + 64, 128 * i:128 * i + 128],
                                        qt_sb[pair][pbase:pbase + 64,
                                                    512 * j + 128 * dd:512 * j + 512],
                                        start=(dd != 3), stop=(dd != 2))
                            else:
                                for n, i in enumerate(trips[t]):
                                    nc.tensor.matmul(
                                        stile[:, 512 * n:512 * n + 512],
                                        kt_sb[pair][pbase:pbase + 64, 128 * i:128 * i + 128],
                                        qt_sb[pair][pbase:pbase + 64, 512 * j:512 * j + 512],
                                        start=True, stop=True)

                        emit_scores(0)
                        if ntrip > 1:
                            emit_scores(1)
                        if j > 0:
                            emit_phase_c(j - 1, [2 * h, 2 * h + 1])
                        opsum = psB.tile([128, 512], F32, name="opsum", tag="acc", bufs=2)
                        for t in range(ntrip):
                            ptt = pb.tile([128, 1536], BF16, name="ptt", tag="ptt", bufs=3)
                            if trips[t] == "diag":
                                # one wide exp covering the 896:1024 gap (cols
                                # there are stale psum, never read by PV)
                                nc.scalar.activation(ptt[:, 0:1408], st_tiles[t][:, 0:1408],
                                                     AF.Exp)
                            else:
                                width = 512 * len(trips[t])
                                nc.scalar.activation(ptt[:, 0:width], st_tiles[t][:, 0:width],
                                                     AF.Exp)
                            if trips[t] == "diag":
                                for dd in range(4):
                                    nc.vector.tensor_mul(
                                        ptt[:, DOFF[dd]:DOFF[dd] + 128],
                                        ptt[:, DOFF[dd]:DOFF[dd] + 128],
                                        cm_sb[:, dd, 128 * dd:128 * dd + 128])
                            if t + 2 < ntrip:
                                emit_scores(t + 2)
                            if trips[t] == "diag":
                                for dd in range(4):
                                    nc.tensor.matmul(
                                        opsum[0:65, 128 * dd:512],
                                        v_sb[:, 4 * j + dd, h, :],
                                        ptt[:, DOFF[dd]:DOFF[dd] + DW[dd]],
                                        start=(j == 0 and dd == 0), stop=(dd == 3))
                            else:
                                for n, i in enumerate(trips[t]):
                                    nc.tensor.matmul(
                                        opsum[0:65, :],
                                        v_sb[:, i, h, :],
                                        ptt[:, 512 * n:512 * n + 512],
                                        start=(t == 0 and n == 0), stop=False)
                        # normalize immediately: by the time the next unit's
                        # phase-C yps needs this opsum's slot, the norm chain
                        # has had a whole unit to drain.
                        emit_norm(h, j, opsum)
                emit_phase_c(NJ - 1, list(range(8)))

    nc.compile()
    return nc


def _run(inputs, trace=False, debug=False, tmpdir=None):
    global _NC
    if _NC is None:
        _NC = _build(debug=debug)
    q = np.asarray(inputs["q"], dtype=np.float32)
    k = np.asarray(inputs["k"], dtype=np.float32)
    v = np.asarray(inputs["v"], dtype=np.float32)
    mask = np.asarray(inputs["mask"])
    w_query = np.asarray(inputs["w_query"], dtype=np.float32)
    w_key = np.asarray(inputs["w_key"], dtype=np.float32)
    w_value = np.asarray(inputs["w_value"], dtype=np.float32)
    w_0 = np.asarray(inputs["w_0"], dtype=np.float32)

    cmask = np.stack([
        np.ascontiguousarray(mask[0, 0, 0:512, 128 * i:128 * i + 128].T)
        for i in range(4)
    ]).astype(ml_dtypes.bfloat16)
    xq_b = [np.ascontiguousarray(q[b].T).astype(np.float16) for b in range(B)]
    xk_b = [np.ascontiguousarray(k[b].T).astype(np.float16) for b in range(B)]
    xv_b = [np.ascontiguousarray(v[b].T).astype(ml_dtypes.bfloat16) for b in range(B)]

    in_maps = []
    for c in range(8):
        b, g = c // 4, c % 4
        sl = slice(256 * g, 256 * g + 256)
        in_maps.append({
            "xq": xq_b[b], "xk": xk_b[b], "xv": xv_b[b],
            "wq": np.ascontiguousarray(w_query[sl, :].T).astype(np.float16),
            "wk": np.ascontiguousarray(w_key[sl, :].T).astype(np.float16),
            "wv": np.ascontiguousarray(w_value[sl, :].T).astype(ml_dtypes.bfloat16),
            "w0": np.ascontiguousarray(w_0[:, sl].T).astype(np.float16),
            "cm": cmask,
        })

    res = run_bass_kernel_spmd(_NC, in_maps, core_ids=list(range(8)), trace=trace,
                               tmpdir=tmpdir)
    y = np.empty((B, S, D), dtype=np.float32)
    for b in range(B):
        acc = res.results[4 * b]["yt"].astype(np.float32)
        for g in range(1, 4):
            acc += res.results[4 * b + g]["yt"].astype(np.float32)
        y[b] = acc.T
    if debug:
        return y, getattr(res, "exec_time_ns", None), res
    return y, getattr(res, "exec_time_ns", None)


def kernel(**inputs):
    return _run(inputs, trace=False)[0]
